# revision 22
# baseline (speedup 1.0000x reference)
"""Bahdanau attention Trainium2 kernel.

Full-input contract: kernel(**inputs) takes the unsharded numpy inputs and
returns (context_vector, attention_weights) matching the fp32 reference.

Strategy: data-parallel over batch B=32 across 8 NeuronCores (4 batches per
core). Per core the encoder is streamed ONCE, transposed (E on partitions,
bf16). The projection matmul consumes it on TensorE; the context reduction
reuses the same resident tiles on VectorE via scalar_tensor_tensor with
accum_out (attn broadcast x encT, accumulated along S), so a second
natural-layout copy is never loaded. The whole pipeline is per-batch so
batch b's softmax/context overlaps batch b+1's projection. Scores live on
SBUF partition 32*b; the length mask is folded into the score PSUM as a
third accumulating matmul; softmax skips max-subtraction since
|scores| <= sum|v| bounds exp well inside fp32.
"""

import numpy as np
import ml_dtypes

import concourse.bacc as bacc
import concourse.mybir as mybir
from concourse import tile
from concourse.bass_utils import run_bass_kernel_spmd
from concourse.masks import make_identity

bf16 = ml_dtypes.bfloat16
F32 = mybir.dt.float32
BF16 = mybir.dt.bfloat16
I32 = mybir.dt.int32

B, S, E, D, A = 32, 2048, 1024, 1024, 256
NCORES = 8
NB = B // NCORES          # batches per core = 4
EC = E // 128             # e chunks = 8
SC4 = 4                   # s chunks of 512
AC = A // 128             # a chunks = 2
ROW = 32                  # batch b lives on partition 32*b


def build_nc():
    nc = bacc.Bacc("TRN2", target_bir_lowering=False, debug=False)

    encT = nc.dram_tensor("encT", (NB, E, S), BF16, kind="ExternalInput")
    encN = nc.dram_tensor("encN", (NB, 512, E), BF16, kind="ExternalInput")
    wencT = nc.dram_tensor("wencT", (E, A), BF16, kind="ExternalInput")
    wdecT = nc.dram_tensor("wdecT", (D, A), BF16, kind="ExternalInput")
    hT = nc.dram_tensor("hT", (D, NB), BF16, kind="ExternalInput")
    vv = nc.dram_tensor("vv", (A, 1), BF16, kind="ExternalInput")
    lens = nc.dram_tensor("lens", (128, 1), I32, kind="ExternalInput")
    ctx_out = nc.dram_tensor("ctx_out", (NB, E), F32, kind="ExternalOutput")
    ctxP_out = nc.dram_tensor("ctxP_out", (NB, E), F32, kind="ExternalOutput")
    attn_out = nc.dram_tensor("attn_out", (NB, S), F32, kind="ExternalOutput")

    with tile.TileContext(nc) as tc:
        with (
            tc.tile_pool(name="consts", bufs=1) as consts,
            tc.tile_pool(name="smx", bufs=1) as smx,
            tc.tile_pool(name="encT_p", bufs=NB) as encT_p,
            tc.tile_pool(name="tanh_p", bufs=1) as tanh_p,
            tc.tile_pool(name="abc_p", bufs=1) as abc_p,
            tc.tile_pool(name="encN_p", bufs=1) as encN_p,
            tc.tile_pool(name="mm1_ps", bufs=1, space="PSUM") as mm1_ps,
            tc.tile_pool(name="sc_ps", bufs=2, space="PSUM") as sc_ps,
            tc.tile_pool(name="bc_ps", bufs=2, space="PSUM") as bc_ps,
        ):
            # ---------- encoder DMAs first: PE start gates on these ----------
            et_tiles = {}
            for b in range(NB):
                et = encT_p.tile([128, EC * S], BF16, name=f"et{b}", tag="et")
                et_tiles[b] = et
            w_sb = []
            for e in range(EC):
                w_t = consts.tile([128, A], BF16, name=f"w_t{e}")
                nc.sync.dma_start(w_t[:], wencT[e * 128:(e + 1) * 128, :])
                w_sb.append(w_t)
            # ---------- small constants (before bulk encoder DMAs) ----------
            wd_sb = []
            for d in range(EC):
                wd_t = consts.tile([128, A], BF16, name=f"wd_t{d}")
                nc.sync.dma_start(wd_t[:], wdecT[d * 128:(d + 1) * 128, :])
                wd_sb.append(wd_t)
            hT_sb = consts.tile([128, EC * NB], BF16)
            nc.sync.dma_start(
                hT_sb[:].rearrange("p (d b) -> p d b", d=EC),
                hT.rearrange("(d p) b -> p d b", d=EC),
            )
            v_sb = consts.tile([128, AC], BF16)
            nc.sync.dma_start(
                v_sb[:].rearrange("p (a one) -> p a one", a=AC),
                vv.rearrange("(a p) one -> p a one", a=AC),
            )
            lens_sb = consts.tile([128, 1], I32)
            nc.sync.dma_start(lens_sb[:], lens[:])
            for e in reversed(range(EC)):
                nc.sync.dma_start(
                    et_tiles[0][:, e * S:(e + 1) * S],
                    encT[0, e * 128:(e + 1) * 128, :],
                )
            for b in range(1, NB):
                for e in range(EC):
                    nc.sync.dma_start(
                        et_tiles[b][:, e * S:(e + 1) * S],
                        encT[b, e * 128:(e + 1) * 128, :],
                    )
            en_tiles = {}
            for b in range(NB):
                en = encN_p.tile([128, 4 * E], BF16, name=f"en{b}", tag=f"en{b}")
                en_tiles[b] = en
                for c in range(4):
                    nc.sync.dma_start(
                        en[:, c * E:(c + 1) * E],
                        encN[b, c * 128:(c + 1) * 128, :],
                    )
            lensf = consts.tile([128, 1], F32)
            nc.vector.tensor_copy(lensf[:], lens_sb[:])
            # selector: sel[:, b*128:(b+1)*128] has ones in row 32b
            sel_sb = consts.tile([128, NB * 128], BF16)
            nc.vector.memset(sel_sb[:], 0.0)
            for b in range(NB):
                nc.vector.memset(
                    sel_sb[ROW * b: ROW * b + 1, b * 128:(b + 1) * 128], 1.0
                )
            ident = consts.tile([128, 128], F32)
            make_identity(nc, ident[:])
            ident_h = consts.tile([128, 128], BF16)
            make_identity(nc, ident_h[:])

            # HAM warm-up: dense dummy matmuls on local (memset) data so the
            # PE clock-gate opens while the encoder DMA is still streaming.
            warm_ps = mm1_ps.tile([128, 512], F32, name="warm_ps", tag="ps0")
            for i in range(160):
                nc.tensor.matmul(
                    warm_ps[:, :128], sel_sb[:, :128], sel_sb[:, :128],
                    start=True, stop=True,
                )

            # ---------- dec_proj = W_dec @ h  -> dec_sb (128, AC*NB) ----------
            dec_sb = consts.tile([128, AC * NB], F32)
            for a in range(AC):
                dps = bc_ps.tile([128, 512], F32, name="dps", tag="bps")
                for d in range(EC):
                    nc.tensor.matmul(
                        dps[:, :NB],
                        wd_sb[d][:, a * 128:(a + 1) * 128],
                        hT_sb[:, d * NB:(d + 1) * NB],
                        start=(d == 0), stop=(d == EC - 1),
                    )
                nc.vector.tensor_copy(dec_sb[:, a * NB:(a + 1) * NB], dps[:, :NB])

            # ---------- additive length mask, batch-rows layout (bf16) ------
            iota_t = smx.tile([128, S], F32, tag="scratchA")
            nc.gpsimd.iota(
                iota_t[:], pattern=[[1, S]], base=0, channel_multiplier=0,
                allow_small_or_imprecise_dtypes=True,
            )
            maskbig = smx.tile([128, S], BF16)
            # (iota < len ? 1 : 0) - 1) * 1e9  -> 0 on valid, -1e9 on invalid
            nc.vector.tensor_scalar(
                out=maskbig[:], in0=iota_t[:], scalar1=lensf[:], scalar2=-1.0,
                op0=mybir.AluOpType.is_lt, op1=mybir.AluOpType.add,
            )
            nc.vector.tensor_scalar(
                out=maskbig[:], in0=maskbig[:], scalar1=1e9, scalar2=None,
                op0=mybir.AluOpType.mult,
            )

            p_sb = smx.tile([128, S], F32, tag="scratchA")
            nc.vector.memset(p_sb[:], 0.0)
            attn_h = smx.tile([128, S], BF16)
            nc.vector.memset(attn_h[:], 0.0)
            sums4 = smx.tile([128, SC4], F32)
            rec = smx.tile([128, 1], F32)
            nc.vector.memset(rec[:], 0.0)
            ctxT_sb = smx.tile([128, NB * EC], F32)
            scr = smx.tile([128, S], BF16)

            # ---------- per-batch pipeline ----------
            for b in range(NB):
                et = et_tiles[b]
                # mm1 + tanh (enc_projT for batch b)
                tanh_tiles = []
                for a in range(AC):
                    tt = tanh_p.tile([128, S], BF16, name=f"tt{a}", tag=f"tt{a}")
                    for ei, e in enumerate(reversed(range(EC))):
                        for sc in range(SC4):
                            if sc == 0 and ei == 0:
                                pss = [mm1_ps.tile([128, 512], F32, name=f"ps{i}", tag=f"ps{i}") for i in range(SC4)]
                            nc.tensor.matmul(
                                pss[sc][:],
                                w_sb[e][:, a * 128:(a + 1) * 128],
                                et[:, e * S + sc * 512: e * S + (sc + 1) * 512],
                                start=(ei == 0), stop=(ei == EC - 1),
                            )
                    bias_ap = dec_sb[:, a * NB + b: a * NB + b + 1]
                    for sc in range(SC4):
                        nc.scalar.activation(
                            tt[:, sc * 512:(sc + 1) * 512], pss[sc][:],
                            mybir.ActivationFunctionType.Tanh,
                            bias=bias_ap, scale=1.0,
                        )
                    tanh_tiles.append(tt)

                # scores (+mask) into PSUM row 32b, then exp+rowsum via ACT
                for sc in range(SC4):
                    sl = slice(sc * 512, (sc + 1) * 512)
                    sps = sc_ps.tile([128, 512], F32, tag="sps")
                    out_ap = sps[ROW * b: ROW * b + 1, :]
                    for a in range(AC):
                        nc.tensor.matmul(
                            out_ap,
                            v_sb[:, a: a + 1],
                            tanh_tiles[a][:, sl],
                            start=(a == 0), stop=False,
                            tile_position=(0, ROW * b),
                        )
                    nc.tensor.matmul(
                        out_ap,
                        sel_sb[:, b * 128: b * 128 + 1],
                        maskbig[:, sl],
                        start=False, stop=True,
                        tile_position=(0, ROW * b),
                    )
                    nc.scalar.activation(
                        p_sb[ROW * b: ROW * b + 1, sl], out_ap,
                        mybir.ActivationFunctionType.Exp,
                        accum_out=sums4[ROW * b: ROW * b + 1, sc: sc + 1],
                    )

                # 1/sum over the 4 chunk partial sums (single lane, tiny)
                ssum = smx.tile([128, 1], F32, tag="ssum")
                nc.vector.tensor_reduce(
                    out=ssum[ROW * b: ROW * b + 1, :],
                    in_=sums4[ROW * b: ROW * b + 1, :],
                    axis=mybir.AxisListType.X, op=mybir.AluOpType.add,
                )
                nc.vector.reciprocal(
                    rec[ROW * b: ROW * b + 1, :], ssum[ROW * b: ROW * b + 1, :]
                )
                rec_ap = rec[ROW * b: ROW * b + 1, 0:1]
                nc.scalar.activation(
                    attn_h[ROW * b: ROW * b + 1, :], p_sb[ROW * b: ROW * b + 1, :],
                    mybir.ActivationFunctionType.Copy, scale=rec_ap,
                )

                # broadcast attn row to all partitions: DMA-hop the row
                # to partition 0 (GPSIMD bcast reads physical partition 0),
                # then GPSIMD partition_broadcast
                arow = abc_p.tile([1, S], BF16, name="arow", tag="arow")
                nc.sync.dma_start(arow[:], attn_h[ROW * b: ROW * b + 1, :])
                abc = abc_p.tile([128, S], BF16, name="abc", tag="abc")
                nc.gpsimd.partition_broadcast(abc[:], arow[:])

                # context: ctxT[:, b*EC+e] = sum_s encT*attn  (fused DVE)
                for e in range(EC):
                    eng = nc.vector
                    out_t = scr
                    eng.scalar_tensor_tensor(
                        out=out_t[:, 512:],
                        in0=et[:, e * S + 512:(e + 1) * S],
                        scalar=1.0,
                        in1=abc[:, 512:],
                        op0=mybir.AluOpType.mult,
                        op1=mybir.AluOpType.mult,
                        accum_out=ctxT_sb[:, b * EC + e: b * EC + e + 1],
                    )

            # attention_weights output: one full-width in-place scale
            nc.scalar.activation(
                p_sb[:], p_sb[:],
                mybir.ActivationFunctionType.Copy, scale=rec[:, 0:1],
            )
            for b in range(NB):
                nc.sync.dma_start(
                    attn_out[b: b + 1, :], p_sb[ROW * b: ROW * b + 1, :]
                )

            # transpose attn (s<512) to s-on-partitions, then PE tail mm2
            attnT = tanh_p.tile([128, 4 * 128], BF16, tag="tt0")
            for c in range(4):
                tp = bc_ps.tile([128, 1024], BF16, name="tp", tag="bps")
                nc.tensor.transpose(
                    tp[:, :128], attn_h[:, c * 128:(c + 1) * 128], ident_h[:]
                )
                nc.scalar.copy(attnT[:, c * 128:(c + 1) * 128], tp[:, :128])
            ctxP = tanh_p.tile([128, E], F32, tag="tt1")
            for b in range(NB):
                en = en_tiles[b]
                for h in range(2):
                    cps = sc_ps.tile([128, 512], F32, name="cps", tag="sps")
                    out_ap = cps[ROW * b: ROW * b + 1, :]
                    for c in range(4):
                        nc.tensor.matmul(
                            out_ap,
                            attnT[:, c * 128 + ROW * b: c * 128 + ROW * b + 1],
                            en[:, c * E + h * 512: c * E + h * 512 + 512],
                            start=(c == 0), stop=(c == 3),
                            tile_position=(0, ROW * b),
                        )
                    nc.scalar.copy(
                        ctxP[ROW * b: ROW * b + 1, h * 512:(h + 1) * 512], out_ap
                    )
                nc.sync.dma_start(
                    ctxP_out[b: b + 1, :], ctxP[ROW * b: ROW * b + 1, :]
                )

            # ---------- gather ctxT (128, NB*EC) -> DRAM ----------
            gps = mm1_ps.tile([128, 512], F32, name="gps", tag="ps0")
            nc.tensor.transpose(gps[:NB * EC, :128], ctxT_sb[:], ident[:])
            ctxg = smx.tile([NB * EC, 128], F32)
            nc.vector.tensor_copy(ctxg[:], gps[:NB * EC, :128])
            nc.sync.dma_start(
                ctx_out.rearrange("b (c x) -> (b c) x", c=EC), ctxg[:]
            )

    nc.compile()
    return nc


def make_in_maps(encoder_outputs, decoder_hidden, input_lengths, W_enc, W_dec, v):
    """Shard + lay out host-side. Returns list of per-core input dicts."""
    enc_b = encoder_outputs.astype(bf16)          # (B, S, E)
    encT_b = np.ascontiguousarray(enc_b.transpose(0, 2, 1))  # (B, E, S)
    wencT = np.ascontiguousarray(W_enc.T).astype(bf16)       # (E, A)
    wdecT = np.ascontiguousarray(W_dec.T).astype(bf16)       # (D, A)
    vvT = np.ascontiguousarray(v.reshape(1, A).T).astype(bf16)  # (A, 1)
    hT_all = decoder_hidden.T.astype(bf16)        # (D, B)

    in_maps = []
    for c in range(NCORES):
        sl = slice(c * NB, (c + 1) * NB)
        lens_exp = np.full((128, 1), S, dtype=np.int32)
        lens_exp[::ROW, 0][:NB] = input_lengths[sl]
        in_maps.append({
            "encT": np.ascontiguousarray(encT_b[sl]),
            "encN": np.ascontiguousarray(enc_b[sl, :512, :]),
            "wencT": wencT,
            "wdecT": wdecT,
            "hT": np.ascontiguousarray(hT_all[:, sl]),
            "vv": vvT,
            "lens": lens_exp,
        })
    return in_maps


_NC_CACHE = None


def kernel(encoder_outputs, decoder_hidden, input_lengths, W_enc, W_dec, v):
    global _NC_CACHE
    if _NC_CACHE is None:
        _NC_CACHE = build_nc()
    nc = _NC_CACHE
    in_maps = make_in_maps(
        encoder_outputs, decoder_hidden, input_lengths, W_enc, W_dec, v
    )
    res = run_bass_kernel_spmd(nc, in_maps, core_ids=list(range(NCORES)))
    ctx = (np.concatenate([r["ctx_out"] for r in res.results], axis=0)
           + np.concatenate([r["ctxP_out"] for r in res.results], axis=0))
    attn = np.concatenate([r["attn_out"] for r in res.results], axis=0)
    return ctx.astype(np.float32), attn.astype(np.float32)


# revision 23
# speedup vs baseline: 1.2026x; 1.2026x over previous
"""Bahdanau attention Trainium2 kernel.

Full-input contract: kernel(**inputs) takes the unsharded numpy inputs and
returns (context_vector, attention_weights) matching the fp32 reference.

Strategy: data-parallel over batch B=32 across 8 NeuronCores (4 batches per
core). Per core the encoder is streamed ONCE, transposed (E on partitions,
bf16). The projection matmul consumes it on TensorE; the context reduction
reuses the same resident tiles on VectorE via scalar_tensor_tensor with
accum_out (attn broadcast x encT, accumulated along S), so a second
natural-layout copy is never loaded. The whole pipeline is per-batch so
batch b's softmax/context overlaps batch b+1's projection. Scores live on
SBUF partition 32*b; the length mask is folded into the score PSUM as a
third accumulating matmul; softmax skips max-subtraction since
|scores| <= sum|v| bounds exp well inside fp32.
"""

import numpy as np
import ml_dtypes

import concourse.bacc as bacc
import concourse.mybir as mybir
from concourse import tile
from concourse.bass_utils import run_bass_kernel_spmd
from concourse.masks import make_identity

bf16 = ml_dtypes.bfloat16
F32 = mybir.dt.float32
BF16 = mybir.dt.bfloat16
I32 = mybir.dt.int32

B, S, E, D, A = 32, 2048, 1024, 1024, 256
NCORES = 8
NB = B // NCORES          # batches per core = 4
EC = E // 128             # e chunks = 8
SC4 = 4                   # s chunks of 512
AC = A // 128             # a chunks = 2
ROW = 32                  # batch b lives on partition 32*b


def build_nc():
    nc = bacc.Bacc("TRN2", target_bir_lowering=False, debug=False)

    encT = nc.dram_tensor("encT", (NB, E, S), BF16, kind="ExternalInput")
    encN = nc.dram_tensor("encN", (NB, 512, E), BF16, kind="ExternalInput")
    wencT = nc.dram_tensor("wencT", (E, A), BF16, kind="ExternalInput")
    wdecT = nc.dram_tensor("wdecT", (D, A), BF16, kind="ExternalInput")
    hT = nc.dram_tensor("hT", (D, NB), BF16, kind="ExternalInput")
    vv = nc.dram_tensor("vv", (A, 1), BF16, kind="ExternalInput")
    lens = nc.dram_tensor("lens", (128, 1), I32, kind="ExternalInput")
    ctx_out = nc.dram_tensor("ctx_out", (NB, E), F32, kind="ExternalOutput")
    ctxP_out = nc.dram_tensor("ctxP_out", (NB, E), F32, kind="ExternalOutput")
    attn_out = nc.dram_tensor("attn_out", (NB, S), F32, kind="ExternalOutput")

    with tile.TileContext(nc) as tc:
        with (
            tc.tile_pool(name="consts", bufs=1) as consts,
            tc.tile_pool(name="smx", bufs=1) as smx,
            tc.tile_pool(name="encT_p", bufs=NB) as encT_p,
            tc.tile_pool(name="tanh_p", bufs=1) as tanh_p,
            tc.tile_pool(name="abc_p", bufs=1) as abc_p,
            tc.tile_pool(name="encN_p", bufs=1) as encN_p,
            tc.tile_pool(name="mm1_ps", bufs=1, space="PSUM") as mm1_ps,
            tc.tile_pool(name="sc_ps", bufs=2, space="PSUM") as sc_ps,
            tc.tile_pool(name="bc_ps", bufs=2, space="PSUM") as bc_ps,
        ):
            # ---------- encoder DMAs first: PE start gates on these ----------
            et_tiles = {}
            for b in range(NB):
                et = encT_p.tile([128, EC * S], BF16, name=f"et{b}", tag="et")
                et_tiles[b] = et
            w_sb = []
            for e in range(EC):
                w_t = consts.tile([128, A], BF16, name=f"w_t{e}")
                nc.sync.dma_start(w_t[:], wencT[e * 128:(e + 1) * 128, :])
                w_sb.append(w_t)
            # ---------- small constants (before bulk encoder DMAs) ----------
            wd_sb = []
            for d in range(EC):
                wd_t = consts.tile([128, A], BF16, name=f"wd_t{d}")
                nc.sync.dma_start(wd_t[:], wdecT[d * 128:(d + 1) * 128, :])
                wd_sb.append(wd_t)
            hT_sb = consts.tile([128, EC * NB], BF16)
            nc.sync.dma_start(
                hT_sb[:].rearrange("p (d b) -> p d b", d=EC),
                hT.rearrange("(d p) b -> p d b", d=EC),
            )
            v_sb = consts.tile([128, AC], BF16)
            nc.sync.dma_start(
                v_sb[:].rearrange("p (a one) -> p a one", a=AC),
                vv.rearrange("(a p) one -> p a one", a=AC),
            )
            lens_sb = consts.tile([128, 1], I32)
            nc.sync.dma_start(lens_sb[:], lens[:])
            for e in reversed(range(EC)):
                nc.sync.dma_start(
                    et_tiles[0][:, e * S:(e + 1) * S],
                    encT[0, e * 128:(e + 1) * 128, :],
                )
            for b in range(1, NB):
                for e in range(EC):
                    nc.sync.dma_start(
                        et_tiles[b][:, e * S:(e + 1) * S],
                        encT[b, e * 128:(e + 1) * 128, :],
                    )
            en_tiles = {}
            for b in range(NB):
                en = encN_p.tile([128, 4 * E], BF16, name=f"en{b}", tag=f"en{b}")
                en_tiles[b] = en
                for c in range(4):
                    nc.sync.dma_start(
                        en[:, c * E:(c + 1) * E],
                        encN[b, c * 128:(c + 1) * 128, :],
                    )
            lensf = consts.tile([128, 1], F32)
            nc.vector.tensor_copy(lensf[:], lens_sb[:])
            # selector: sel[:, b*128:(b+1)*128] has ones in row 32b
            sel_sb = consts.tile([128, NB * 128], BF16)
            nc.vector.memset(sel_sb[:], 0.0)
            for b in range(NB):
                nc.vector.memset(
                    sel_sb[ROW * b: ROW * b + 1, b * 128:(b + 1) * 128], 1.0
                )
            ident = consts.tile([128, 128], F32)
            make_identity(nc, ident[:])
            ident_h = consts.tile([128, 128], BF16)
            make_identity(nc, ident_h[:])

            # HAM warm-up: dense dummy matmuls on local (memset) data so the
            # PE clock-gate opens while the encoder DMA is still streaming.
            warm_ps = mm1_ps.tile([128, 512], F32, name="warm_ps", tag="ps0")
            for i in range(160):
                nc.tensor.matmul(
                    warm_ps[:, :128], sel_sb[:, :128], sel_sb[:, :128],
                    start=True, stop=True,
                )

            # ---------- dec_proj = W_dec @ h  -> dec_sb (128, AC*NB) ----------
            dec_sb = consts.tile([128, AC * NB], F32)
            for a in range(AC):
                dps = bc_ps.tile([128, 512], F32, name="dps", tag="bps")
                for d in range(EC):
                    nc.tensor.matmul(
                        dps[:, :NB],
                        wd_sb[d][:, a * 128:(a + 1) * 128],
                        hT_sb[:, d * NB:(d + 1) * NB],
                        start=(d == 0), stop=(d == EC - 1),
                    )
                nc.vector.tensor_copy(dec_sb[:, a * NB:(a + 1) * NB], dps[:, :NB])

            # ---------- additive length mask, batch-rows layout (bf16) ------
            iota_t = smx.tile([128, S], F32, tag="scratchA")
            nc.gpsimd.iota(
                iota_t[:], pattern=[[1, S]], base=0, channel_multiplier=0,
                allow_small_or_imprecise_dtypes=True,
            )
            maskbig = smx.tile([128, S], BF16)
            # (iota < len ? 1 : 0) - 1) * 1e9  -> 0 on valid, -1e9 on invalid
            nc.vector.tensor_scalar(
                out=maskbig[:], in0=iota_t[:], scalar1=lensf[:], scalar2=-1.0,
                op0=mybir.AluOpType.is_lt, op1=mybir.AluOpType.add,
            )
            nc.vector.tensor_scalar(
                out=maskbig[:], in0=maskbig[:], scalar1=1e9, scalar2=None,
                op0=mybir.AluOpType.mult,
            )

            p_sb = smx.tile([128, S], F32, tag="scratchA")
            nc.vector.memset(p_sb[:], 0.0)
            attn_h = smx.tile([128, S], BF16)
            nc.vector.memset(attn_h[:], 0.0)
            sums4 = smx.tile([128, SC4], F32)
            rec = smx.tile([128, 1], F32)
            nc.vector.memset(rec[:], 0.0)
            ctxT_sb = smx.tile([128, NB * EC], F32)
            scr = smx.tile([128, S], BF16)

            # ---------- per-batch pipeline ----------
            for b in range(NB):
                et = et_tiles[b]
                # mm1 + tanh (enc_projT for batch b)
                tanh_tiles = []
                for a in range(AC):
                    tt = tanh_p.tile([128, S], BF16, name=f"tt{a}", tag=f"tt{a}")
                    for ei, e in enumerate(reversed(range(EC))):
                        for sc in range(SC4):
                            if sc == 0 and ei == 0:
                                pss = [mm1_ps.tile([128, 512], F32, name=f"ps{i}", tag=f"ps{i}") for i in range(SC4)]
                            nc.tensor.matmul(
                                pss[sc][:],
                                w_sb[e][:, a * 128:(a + 1) * 128],
                                et[:, e * S + sc * 512: e * S + (sc + 1) * 512],
                                start=(ei == 0), stop=(ei == EC - 1),
                            )
                    bias_ap = dec_sb[:, a * NB + b: a * NB + b + 1]
                    for sc in range(SC4):
                        nc.scalar.activation(
                            tt[:, sc * 512:(sc + 1) * 512], pss[sc][:],
                            mybir.ActivationFunctionType.Tanh,
                            bias=bias_ap, scale=1.0,
                        )
                    tanh_tiles.append(tt)

                # scores (+mask) into PSUM row 32b, then exp+rowsum via ACT
                for sc in range(SC4):
                    sl = slice(sc * 512, (sc + 1) * 512)
                    sps = sc_ps.tile([128, 512], F32, tag="sps")
                    out_ap = sps[ROW * b: ROW * b + 1, :]
                    for a in range(AC):
                        nc.tensor.matmul(
                            out_ap,
                            v_sb[:, a: a + 1],
                            tanh_tiles[a][:, sl],
                            start=(a == 0), stop=False,
                            tile_position=(0, ROW * b),
                        )
                    nc.tensor.matmul(
                        out_ap,
                        sel_sb[:, b * 128: b * 128 + 1],
                        maskbig[:, sl],
                        start=False, stop=True,
                        tile_position=(0, ROW * b),
                    )
                    nc.scalar.activation(
                        p_sb[ROW * b: ROW * b + 1, sl], out_ap,
                        mybir.ActivationFunctionType.Exp,
                        accum_out=sums4[ROW * b: ROW * b + 1, sc: sc + 1],
                    )

                # 1/sum over the 4 chunk partial sums (single lane, tiny)
                ssum = smx.tile([128, 1], F32, tag="ssum")
                nc.vector.tensor_reduce(
                    out=ssum[ROW * b: ROW * b + 1, :],
                    in_=sums4[ROW * b: ROW * b + 1, :],
                    axis=mybir.AxisListType.X, op=mybir.AluOpType.add,
                )
                nc.vector.reciprocal(
                    rec[ROW * b: ROW * b + 1, :], ssum[ROW * b: ROW * b + 1, :]
                )
                rec_ap = rec[ROW * b: ROW * b + 1, 0:1]
                nc.scalar.activation(
                    attn_h[ROW * b: ROW * b + 1, :], p_sb[ROW * b: ROW * b + 1, :],
                    mybir.ActivationFunctionType.Copy, scale=rec_ap,
                )

                # broadcast attn row to all partitions (selector matmul)
                abc = abc_p.tile([128, S], BF16, name="abc", tag="abc")
                for q in range(SC4):
                    bps = bc_ps.tile([128, 512], F32, tag="bps")
                    nc.tensor.matmul(
                        bps[:], sel_sb[:, b * 128:(b + 1) * 128],
                        attn_h[:, q * 512:(q + 1) * 512],
                        start=True, stop=True,
                    )
                    nc.scalar.copy(abc[:, q * 512:(q + 1) * 512], bps[:])

                # context: ctxT[:, b*EC+e] = sum_s encT*attn  (fused DVE)
                for e in range(EC):
                    eng = nc.vector
                    out_t = scr
                    eng.scalar_tensor_tensor(
                        out=out_t[:, 512:],
                        in0=et[:, e * S + 512:(e + 1) * S],
                        scalar=1.0,
                        in1=abc[:, 512:],
                        op0=mybir.AluOpType.mult,
                        op1=mybir.AluOpType.mult,
                        accum_out=ctxT_sb[:, b * EC + e: b * EC + e + 1],
                    )

            # attention_weights output: one full-width in-place scale
            nc.scalar.activation(
                p_sb[:], p_sb[:],
                mybir.ActivationFunctionType.Copy, scale=rec[:, 0:1],
            )
            for b in range(NB):
                nc.sync.dma_start(
                    attn_out[b: b + 1, :], p_sb[ROW * b: ROW * b + 1, :]
                )

            # transpose attn (s<512) to s-on-partitions, then PE tail mm2
            attnT = tanh_p.tile([128, 4 * 128], BF16, tag="tt0")
            for c in range(4):
                tp = bc_ps.tile([128, 1024], BF16, name="tp", tag="bps")
                nc.tensor.transpose(
                    tp[:, :128], attn_h[:, c * 128:(c + 1) * 128], ident_h[:]
                )
                nc.scalar.copy(attnT[:, c * 128:(c + 1) * 128], tp[:, :128])
            ctxP = tanh_p.tile([128, E], F32, tag="tt1")
            for b in range(NB):
                en = en_tiles[b]
                for h in range(2):
                    cps = sc_ps.tile([128, 512], F32, name="cps", tag="sps")
                    out_ap = cps[ROW * b: ROW * b + 1, :]
                    for c in range(4):
                        nc.tensor.matmul(
                            out_ap,
                            attnT[:, c * 128 + ROW * b: c * 128 + ROW * b + 1],
                            en[:, c * E + h * 512: c * E + h * 512 + 512],
                            start=(c == 0), stop=(c == 3),
                            tile_position=(0, ROW * b),
                        )
                    nc.scalar.copy(
                        ctxP[ROW * b: ROW * b + 1, h * 512:(h + 1) * 512], out_ap
                    )
                nc.sync.dma_start(
                    ctxP_out[b: b + 1, :], ctxP[ROW * b: ROW * b + 1, :]
                )

            # ---------- gather ctxT (128, NB*EC) -> DRAM ----------
            gps = mm1_ps.tile([128, 512], F32, name="gps", tag="ps0")
            nc.tensor.transpose(gps[:NB * EC, :128], ctxT_sb[:], ident[:])
            ctxg = smx.tile([NB * EC, 128], F32)
            nc.vector.tensor_copy(ctxg[:], gps[:NB * EC, :128])
            nc.sync.dma_start(
                ctx_out.rearrange("b (c x) -> (b c) x", c=EC), ctxg[:]
            )

    nc.compile()
    return nc


def make_in_maps(encoder_outputs, decoder_hidden, input_lengths, W_enc, W_dec, v):
    """Shard + lay out host-side. Returns list of per-core input dicts."""
    enc_b = encoder_outputs.astype(bf16)          # (B, S, E)
    encT_b = np.ascontiguousarray(enc_b.transpose(0, 2, 1))  # (B, E, S)
    wencT = np.ascontiguousarray(W_enc.T).astype(bf16)       # (E, A)
    wdecT = np.ascontiguousarray(W_dec.T).astype(bf16)       # (D, A)
    vvT = np.ascontiguousarray(v.reshape(1, A).T).astype(bf16)  # (A, 1)
    hT_all = decoder_hidden.T.astype(bf16)        # (D, B)

    in_maps = []
    for c in range(NCORES):
        sl = slice(c * NB, (c + 1) * NB)
        lens_exp = np.full((128, 1), S, dtype=np.int32)
        lens_exp[::ROW, 0][:NB] = input_lengths[sl]
        in_maps.append({
            "encT": np.ascontiguousarray(encT_b[sl]),
            "encN": np.ascontiguousarray(enc_b[sl, :512, :]),
            "wencT": wencT,
            "wdecT": wdecT,
            "hT": np.ascontiguousarray(hT_all[:, sl]),
            "vv": vvT,
            "lens": lens_exp,
        })
    return in_maps


_NC_CACHE = None


def kernel(encoder_outputs, decoder_hidden, input_lengths, W_enc, W_dec, v):
    global _NC_CACHE
    if _NC_CACHE is None:
        _NC_CACHE = build_nc()
    nc = _NC_CACHE
    in_maps = make_in_maps(
        encoder_outputs, decoder_hidden, input_lengths, W_enc, W_dec, v
    )
    res = run_bass_kernel_spmd(nc, in_maps, core_ids=list(range(NCORES)))
    ctx = (np.concatenate([r["ctx_out"] for r in res.results], axis=0)
           + np.concatenate([r["ctxP_out"] for r in res.results], axis=0))
    attn = np.concatenate([r["attn_out"] for r in res.results], axis=0)
    return ctx.astype(np.float32), attn.astype(np.float32)
